# revision 16
# baseline (speedup 1.0000x reference)
"""Trainium2 Bass kernel for GaussianSplatting-style field + gradient evaluation.

Math:
  field[m]  = sum_n s(r_mn) * act[n]
  grad_x[m] = sum_n (-beta) s(1-s) (dx/r) act[n]      (same for y)
  s = sigmoid(-beta (r - sigma)) = 0.5 + 0.5 tanh((beta sigma - beta r)/2)
  r = sqrt(dx^2 + dy^2 + eps^2)

Device decomposition (per core, data-parallel over M):
  u[n, m] = |q_m - p_n|^2 + eps^2 computed by a K=3 TensorE matmul
  (rows qx, qy, |q|^2 against weights -2px, -2py, 1 plus a per-n bias col),
  then ACT sqrt -> r, DVE reciprocal_approx_fast -> 1/r, ACT tanh -> h,
  ACT square -> h^2, DVE affine_mul_reduce -> t = 0.25(1-h^2)/r.
  Reductions over n are K=128 matmuls: field stream (h against 0.5*act) and
  grad stream (t against -beta*act*{1, px, py}), accumulated in one PSUM
  [4, 512] block per chunk of 512 queries. Final fixup:
  grad_x = qx*U - Vx, field = F + 0.5*sum(act), computed on 128-partition
  tiles, written as out[3, MS].

Numerical safety: the K=3 expansion of u catastrophically cancels for
near-coincident pairs. Host routes every query whose nearest neuron is
closer than sqrt(1e-4) into the leading "exact" chunks, where dx^2 is
computed per-pair as ACT Square(qx_broadcast - px_col) (no cancellation).
Dense chunks add a U0=2e-7 guard to the sqrt bias so u stays positive.
"""

import os
import sys

sys.path.insert(0, "/opt/trn_rl_repo")

import numpy as np

LAST_RESULT = None  # BassKernelResults of the most recent kernel() call

NCORES = 8
M_TOTAL = 262144
MS = M_TOTAL // NCORES  # 32768 queries per core
N = 256
CH = 512                # queries per chunk
NCH = MS // CH          # 64 chunks per core
GROUP = 16              # chunks per ACT-table-set phase group
FIXF = MS // 128        # 256: fixup tile free dim
EPS2 = 1e-16
U0 = 2e-7               # dense-path guard on u
R12 = 1e-4              # squared-distance threshold for exact-path routing

_cache = {}


def _build_program(ne):
    """Build the per-core Bass program; ne = number of exact-path chunks."""
    from contextlib import ExitStack

    import concourse.bass as bass
    import concourse.tile as tile
    from concourse import bacc, mybir

    f32 = mybir.dt.float32
    AF = mybir.ActivationFunctionType
    ALU = mybir.AluOpType

    nc = bacc.Bacc("TRN2", target_bir_lowering=False, debug=False)
    q_d = nc.dram_tensor("q", [MS, 2], f32, kind="ExternalInput")
    pos_d = nc.dram_tensor("positions", [2 * N], f32, kind="ExternalInput")
    act_d = nc.dram_tensor("activities", [N], f32, kind="ExternalInput")
    par_d = nc.dram_tensor("params", [1, 8], f32, kind="ExternalInput")
    out_d = nc.dram_tensor("out", [3, MS], f32, kind="ExternalOutput")
    qprep_d = nc.dram_tensor("qprep", [3, MS], f32)
    stage_d = nc.dram_tensor("stage", [4, MS], f32)

    with tile.TileContext(nc) as tc:
        with ExitStack() as ctx:
            v = nc.vector
            sc = nc.scalar

            singles = ctx.enter_context(tc.tile_pool(name="singles", bufs=1))
            prep = ctx.enter_context(tc.tile_pool(name="prep", bufs=4))
            qpool = ctx.enter_context(tc.tile_pool(name="qpool", bufs=3))
            rstore = ctx.enter_context(tc.tile_pool(name="rstore", bufs=2 * GROUP + 2))
            work = ctx.enter_context(tc.tile_pool(name="work", bufs=3))
            upsum = ctx.enter_context(tc.tile_pool(name="upsum", bufs=4, space="PSUM"))
            redps = ctx.enter_context(tc.tile_pool(name="redps", bufs=2, space="PSUM"))
            ppsum = ctx.enter_context(tc.tile_pool(name="ppsum", bufs=1, space="PSUM"))

            # ---------- prep: load small tensors ----------
            q_T = singles.tile([128, 512], f32)          # all shard queries
            nc.sync.dma_start(q_T[:], q_d.ap().rearrange("(p f) c -> p (f c)", p=128))
            posv = singles.tile([128, 4], f32)           # x_e y_e x_o y_o
            nc.sync.dma_start(posv[:], pos_d.ap().rearrange("(p c) -> p c", p=128))
            acol = singles.tile([128, 2], f32)           # act_e act_o
            nc.sync.dma_start(acol[:], act_d.ap().rearrange("(p c) -> p c", p=128))
            parb = singles.tile([128, 8], f32)           # params broadcast
            nc.sync.dma_start(parb[:], par_d.ap().to_broadcast([128, 8]))

            tc.strict_bb_all_engine_barrier()

            axc, ayc = parb[:, 0:1], parb[:, 1:2]
            txc, tyc = parb[:, 2:3], parb[:, 3:4]
            sgc, btc = parb[:, 4:5], parb[:, 5:6]

            # ---------- neuron columns: affine positions ----------
            # px = (x - 0.5)*ax + (tx + 0.5)
            cols = singles.tile([128, 16], f32)
            pxc = [cols[:, 0:1], cols[:, 1:2]]   # even, odd neurons
            pyc = [cols[:, 2:3], cols[:, 3:4]]
            for nt in range(2):
                v.tensor_scalar(pxc[nt], posv[:, 2 * nt : 2 * nt + 1], -0.5, axc,
                                ALU.add, ALU.mult)
                v.tensor_scalar(pxc[nt], pxc[nt], 0.5, None, ALU.add)
                v.tensor_tensor(out=pxc[nt], in0=pxc[nt], in1=txc, op=ALU.add)
                v.tensor_scalar(pyc[nt], posv[:, 2 * nt + 1 : 2 * nt + 2], -0.5, ayc,
                                ALU.add, ALU.mult)
                v.tensor_scalar(pyc[nt], pyc[nt], 0.5, None, ALU.add)
                v.tensor_tensor(out=pyc[nt], in0=pyc[nt], in1=tyc, op=ALU.add)

            # dense sqrt bias = px^2 + py^2 + eps^2 + U0 ; exact Square bias = -px
            biasd = [cols[:, 4:5], cols[:, 5:6]]
            nxc = [cols[:, 6:7], cols[:, 7:8]]
            nyc = [cols[:, 8:9], cols[:, 9:10]]
            tmp = prep.tile([128, 2], f32)
            for nt in range(2):
                v.tensor_tensor(out=tmp[:, 0:1], in0=pxc[nt], in1=pxc[nt], op=ALU.mult)
                v.tensor_tensor(out=tmp[:, 1:2], in0=pyc[nt], in1=pyc[nt], op=ALU.mult)
                v.tensor_tensor(out=biasd[nt], in0=tmp[:, 0:1], in1=tmp[:, 1:2],
                                op=ALU.add)
                v.tensor_scalar(biasd[nt], biasd[nt], float(EPS2 + U0), None, ALU.add)
                v.tensor_scalar(nxc[nt], pxc[nt], -1.0, None, ALU.mult)
                v.tensor_scalar(nyc[nt], pyc[nt], -1.0, None, ALU.mult)

            # tanh arg: h = tanh(thsc * r + thbc), thsc = -beta/2, thbc = beta*sigma/2
            thsc, thbc = cols[:, 10:11], cols[:, 11:12]
            v.tensor_scalar(thsc, btc, -0.5, None, ALU.mult)
            v.tensor_tensor(out=thbc, in0=btc, in1=sgc, op=ALU.mult)
            v.tensor_scalar(thbc, thbc, 0.5, None, ALU.mult)

            # ---------- reduce weights W8h/W8t [128, 4] per n-tile ----------
            w8h = [singles.tile([128, 32], f32, name=f"w8h{i}", tag=f"w8h{i}") for i in range(2)]
            w8t = [singles.tile([128, 32], f32, name=f"w8t{i}", tag=f"w8t{i}") for i in range(2)]
            nba = prep.tile([128, 2], f32)  # -beta * act per n-tile
            for nt in range(2):
                v.memset(w8h[nt][:], 0.0)
                v.memset(w8t[nt][:], 0.0)
                v.tensor_scalar(w8h[nt][:, 0:1], acol[:, nt : nt + 1], 0.5, None,
                                ALU.mult)
                v.tensor_tensor(out=nba[:, nt : nt + 1], in0=acol[:, nt : nt + 1],
                                in1=btc, op=ALU.mult)
                v.tensor_scalar(nba[:, nt : nt + 1], nba[:, nt : nt + 1], -1.0, None,
                                ALU.mult)
                v.tensor_copy(out=w8t[nt][:, 1:2], in_=nba[:, nt : nt + 1])
                v.tensor_tensor(out=w8t[nt][:, 2:3], in0=nba[:, nt : nt + 1],
                                in1=pxc[nt], op=ALU.mult)
                v.tensor_tensor(out=w8t[nt][:, 3:4], in0=nba[:, nt : nt + 1],
                                in1=pyc[nt], op=ALU.mult)

            # ---------- W4 lhsT [3, 128] per n-tile (transpose via DVE 32x32) ----------
            w4 = [singles.tile([3, 128], f32, name=f"w4{i}", tag=f"w4{i}") for i in range(2)]
            ctile = prep.tile([128, 32], f32)
            ttile = prep.tile([128, 32], f32)
            for nt in range(2):
                v.memset(ctile[:], 0.0)
                v.tensor_scalar(ctile[:, 0:1], pxc[nt], -2.0, None, ALU.mult)
                v.tensor_scalar(ctile[:, 1:2], pyc[nt], -2.0, None, ALU.mult)
                v.memset(ctile[:, 2:3], 1.0)
                v.transpose(out=ttile[:], in_=ctile[:])
                for b in range(4):
                    v.tensor_copy(out=w4[nt][0:3, 32 * b : 32 * b + 32],
                                  in_=ttile[32 * b : 32 * b + 3, 0:32])

            # exact-path broadcast lhsT: [3, 128] rows: x-> [1,0,0], y-> [0,1,0]
            # (row 1 can't be memset directly: compute ops must start at a
            # 32-aligned partition, so build columns and DVE-transpose.)
            exl = [singles.tile([3, 128], f32, name=f"exl{i}", tag=f"exl{i}") for i in range(2)]
            for i in range(2):
                v.memset(ctile[:], 0.0)
                v.memset(ctile[:, i : i + 1], 1.0)
                v.transpose(out=ttile[:], in_=ctile[:])
                for b in range(4):
                    v.tensor_copy(out=exl[i][0:3, 32 * b : 32 * b + 32],
                                  in_=ttile[32 * b : 32 * b + 3, 0:32])

            tc.strict_bb_all_engine_barrier()

            # ---------- 0.5 * sum(act) broadcast column ----------
            onesc = singles.tile([128, 1], f32)
            v.memset(onesc[:], 1.0)
            onesr = singles.tile([1, 128], f32)
            v.memset(onesr[:], 1.0)
            psA = ppsum.tile([128, 2], f32)
            nc.tensor.matmul(psA[0:1, 0:2], onesc[:], acol[:], start=True, stop=True)
            a2 = prep.tile([1, 2], f32)
            v.tensor_copy(out=a2[:], in_=psA[0:1, 0:2])
            halfA = prep.tile([1, 1], f32)
            v.tensor_tensor(out=halfA[:], in0=a2[:, 0:1], in1=a2[:, 1:2], op=ALU.add)
            v.tensor_scalar(halfA[:], halfA[:], 0.5, None, ALU.mult)
            psB = ppsum.tile([128, 1], f32)
            nc.tensor.matmul(psB[:, 0:1], onesr[:], halfA[:], start=True, stop=True)
            halfAc = singles.tile([128, 1], f32)
            v.tensor_copy(out=halfAc[:], in_=psB[:, 0:1])

            # ---------- qprep rows: qx, qy, |q|^2 ----------
            qxv = q_T[:, 0:512:2]  # [128, 256] strided views
            qyv = q_T[:, 1:512:2]
            qd = prep.tile([128, 256], f32, tag="qd")
            qprep_r = qprep_d.ap().rearrange("c (p f) -> c p f", p=128)
            v.tensor_copy(out=qd[:], in_=qxv)
            nc.sync.dma_start(qprep_r[0], qd[:])
            qd2 = prep.tile([128, 256], f32, tag="qd")
            v.tensor_copy(out=qd2[:], in_=qyv)
            nc.sync.dma_start(qprep_r[1], qd2[:])
            z1 = prep.tile([128, 256], f32, tag="z")
            v.tensor_tensor(out=z1[:], in0=qxv, in1=qxv, op=ALU.mult)
            z2 = prep.tile([128, 256], f32, tag="z")
            v.tensor_tensor(out=z2[:], in0=qyv, in1=qyv, op=ALU.mult)
            q2t = prep.tile([128, 256], f32, tag="z")
            v.tensor_tensor(out=q2t[:], in0=z1[:], in1=z2[:], op=ALU.add)
            nc.sync.dma_start(qprep_r[2], q2t[:])

            tc.strict_bb_all_engine_barrier()

            # ---------- main loop ----------
            stage_ap = stage_d.ap()
            qprep_ap = qprep_d.ap()
            ngroups = NCH // GROUP
            for g in range(ngroups):
                chunks = range(g * GROUP, (g + 1) * GROUP)
                # phase A: sqrt-set ACT ops, store r tiles
                r_tiles = {}
                q_tiles = {}
                for c in chunks:
                    qt = qpool.tile([3, CH], f32)
                    nc.sync.dma_start(qt[:], qprep_ap[:, c * CH : (c + 1) * CH])
                    q_tiles[c] = qt
                    for nt in range(2):
                        rt = rstore.tile([128, CH], f32, tag="rstore")
                        if c < ne:
                            psx = upsum.tile([128, CH], f32, tag="u")
                            nc.tensor.matmul(psx[:], exl[0][:], qt[:], start=True,
                                             stop=True)
                            psy = upsum.tile([128, CH], f32, tag="u")
                            nc.tensor.matmul(psy[:], exl[1][:], qt[:], start=True,
                                             stop=True)
                            dx2 = work.tile([128, CH], f32, tag="dx2")
                            sc.activation(dx2[:], psx[:], AF.Square, bias=nxc[nt])
                            dy2 = work.tile([128, CH], f32, tag="dy2")
                            sc.activation(dy2[:], psy[:], AF.Square, bias=nyc[nt])
                            uu = work.tile([128, CH], f32, tag="uu")
                            v.affine_then_add(uu[:], dx2[:], dy2[:], 1.0, float(EPS2))
                            sc.activation(rt[:], uu[:], AF.Sqrt)
                        else:
                            psu = upsum.tile([128, CH], f32, tag="u")
                            nc.tensor.matmul(psu[:], w4[nt][:], qt[:], start=True,
                                             stop=True)
                            sc.activation(rt[:], psu[:], AF.Sqrt, bias=biasd[nt])
                        r_tiles[(c, nt)] = rt
                # phase B: tanh-set ACT ops + reductions.
                # 4 chunks share one PSUM bank at partitions 0/32/64/96
                # (matmul col tile_position), copied once to SBUF for the DMA.
                for cq in range(GROUP // 4):
                    red4 = redps.tile([128, CH], f32)
                    quad = [g * GROUP + 4 * cq + j for j in range(4)]
                    for j, c in enumerate(quad):
                        rslice = red4[32 * j : 32 * j + 4, :]
                        rinit = red4[32 * j : 32 * j + 32, :]
                        for nt in range(2):
                            rt = r_tiles[(c, nt)]
                            rs = work.tile([128, CH], f32, tag="rs")
                            v.reciprocal_approx_fast(out=rs[:], in_=rt[:])
                            h = work.tile([128, CH], f32, tag="h")
                            sc.activation(h[:], rt[:], AF.Tanh, bias=thbc, scale=thsc)
                            h2 = work.tile([128, CH], f32, tag="h2")
                            sc.activation(h2[:], h[:], AF.Square)
                            t = work.tile([128, CH], f32, tag="t")
                            amr_acc = work.tile([128, 1], f32, tag="amracc")
                            v.affine_mul_reduce(out=t[:], accum_out=amr_acc[:],
                                                in0=h2[:], in1=rs[:],
                                                scale=-0.25, bias=0.25)
                            nc.tensor.matmul(rinit, w8h[nt][:], h[:],
                                             start=(nt == 0), stop=False,
                                             tile_position=(0, 32 * j))
                            nc.tensor.matmul(rinit, w8t[nt][:], t[:],
                                             start=False, stop=(nt == 1),
                                             tile_position=(0, 32 * j))
                    redsb = work.tile([128, CH], f32, tag="redsb")
                    v.tensor_copy(out=redsb[:], in_=red4[:])
                    for j, c in enumerate(quad):
                        nc.sync.dma_start(stage_ap[:, c * CH : (c + 1) * CH],
                                          redsb[32 * j : 32 * j + 4, :])

            tc.strict_bb_all_engine_barrier()

            # ---------- fixup ----------
            stage_r = stage_d.ap().rearrange("c (p f) -> c p f", p=128)
            out_r = out_d.ap().rearrange("c (p f) -> c p f", p=128)
            ft = work.tile([128, FIXF], f32, tag="ft")
            ut = work.tile([128, FIXF], f32, tag="ut")
            vxt = work.tile([128, FIXF], f32, tag="vxt")
            vyt = work.tile([128, FIXF], f32, tag="vyt")
            nc.sync.dma_start(ft[:], stage_r[0])
            nc.sync.dma_start(ut[:], stage_r[1])
            nc.sync.dma_start(vxt[:], stage_r[2])
            nc.sync.dma_start(vyt[:], stage_r[3])
            fo = work.tile([128, FIXF], f32, tag="fo")
            v.tensor_scalar(fo[:], ft[:], halfAc[:], None, ALU.add)
            nc.sync.dma_start(out_r[0], fo[:])
            gx = work.tile([128, FIXF], f32, tag="gx")
            v.tensor_tensor(out=gx[:], in0=qxv, in1=ut[:], op=ALU.mult)
            v.tensor_tensor(out=gx[:], in0=gx[:], in1=vxt[:], op=ALU.subtract)
            nc.sync.dma_start(out_r[1], gx[:])
            gy = work.tile([128, FIXF], f32, tag="gy")
            v.tensor_tensor(out=gy[:], in0=qyv, in1=ut[:], op=ALU.mult)
            v.tensor_tensor(out=gy[:], in0=gy[:], in1=vyt[:], op=ALU.subtract)
            nc.sync.dma_start(out_r[2], gy[:])

    nc.compile()
    return nc


def _get_program(ne):
    if ne not in _cache:
        _cache[ne] = _build_program(ne)
    return _cache[ne]


_runner_cache = {}


def _get_runner(ne):
    """Persistent jitted 8-core executor for the program (avoids re-jitting
    on every kernel() call). Mirrors bass2jax.run_bass_via_pjrt's multi-core
    path."""
    if ne in _runner_cache:
        return _runner_cache[ne]
    import jax
    from jax.sharding import Mesh, PartitionSpec
    try:
        from jax.experimental.shard_map import shard_map
    except ImportError:
        from jax.shard_map import shard_map
    from concourse import mybir
    from concourse.bass2jax import (
        _bass_exec_p,
        install_neuronx_cc_hook,
        partition_id_tensor,
    )

    nc = _get_program(ne)
    install_neuronx_cc_hook()
    partition_name = nc.partition_id_tensor.name if nc.partition_id_tensor else None
    in_names, out_names, out_avals, zero_outs = [], [], [], []
    for alloc in nc.m.functions[0].allocations:
        if not isinstance(alloc, mybir.MemoryLocationSet):
            continue
        name = alloc.memorylocations[0].name
        if alloc.kind == "ExternalInput":
            if name != partition_name:
                in_names.append(name)
        elif alloc.kind == "ExternalOutput":
            shape = tuple(alloc.tensor_shape)
            dtype = mybir.dt.np(alloc.dtype)
            out_avals.append(jax.core.ShapedArray(shape, dtype))
            out_names.append(name)
            zero_outs.append(np.zeros((NCORES * shape[0], *shape[1:]), dtype))
    n_params = len(in_names)
    all_names = list(in_names) + list(out_names)
    if partition_name is not None:
        all_names.append(partition_name)

    def _body(*args):
        operands = list(args)
        if partition_name is not None:
            operands.append(partition_id_tensor())
        outs = _bass_exec_p.bind(
            *operands,
            out_avals=tuple(out_avals),
            in_names=tuple(all_names),
            out_names=tuple(out_names),
            lowering_input_output_aliases=(),
            sim_require_finite=True,
            sim_require_nnan=True,
            nc=nc,
        )
        return tuple(outs)

    devices = jax.devices()[:NCORES]
    mesh = Mesh(np.asarray(devices), ("core",))
    n_outs = len(out_names)
    sharded = jax.jit(
        shard_map(_body, mesh=mesh,
                  in_specs=(PartitionSpec("core"),) * (n_params + n_outs),
                  out_specs=(PartitionSpec("core"),) * n_outs,
                  check_rep=False),
        donate_argnums=tuple(range(n_params, n_params + n_outs)),
        keep_unused=True,
    )

    def run(in_maps):
        concat_in = [
            np.concatenate([np.asarray(m[name]) for m in in_maps], axis=0)
            for name in in_names
        ]
        out_arrs = sharded(*concat_in, *[z.copy() for z in zero_outs])
        return [
            {name: np.asarray(out_arrs[i]).reshape(NCORES, *out_avals[i].shape)[c]
             for i, name in enumerate(out_names)}
            for c in range(NCORES)
        ]

    run.sharded = sharded
    run.in_names = in_names
    run.zero_outs = zero_outs
    _runner_cache[ne] = run
    return run


_bench_state = {}


def _bench(iters=12):
    """Time steady-state executions of the last kernel() invocation's inputs.
    Returns list of per-call wall seconds (includes dispatch overhead)."""
    import time as _time

    run = _bench_state["run"]
    in_maps = _bench_state["in_maps"]
    concat_in = [
        np.concatenate([np.asarray(m[name]) for m in in_maps], axis=0)
        for name in run.in_names
    ]
    times = []
    for _ in range(iters):
        zeros = [z.copy() for z in run.zero_outs]
        t0 = _time.perf_counter()
        out = run.sharded(*concat_in, *zeros)
        jax.block_until_ready(out)
        times.append(_time.perf_counter() - t0)
    return times


import jax  # noqa: E402  (used by _bench)


def kernel(positions, activities, query_points,
           affine_ax, affine_ay, affine_tx, affine_ty, sigma, beta):
    positions = np.asarray(positions, dtype=np.float32)
    activities = np.asarray(activities, dtype=np.float32)
    query_points = np.asarray(query_points, dtype=np.float32)
    f32 = np.float32
    axv, ayv = f32(affine_ax), f32(affine_ay)
    txv, tyv = f32(affine_tx), f32(affine_ty)
    sgv, btv = f32(sigma), f32(beta)
    M = query_points.shape[0]
    assert M == M_TOTAL

    # host: route near-pair queries into exact chunks (selection only)
    px = axv * (positions[:, 0] - f32(0.5)) + txv + f32(0.5)
    py = ayv * (positions[:, 1] - f32(0.5)) + tyv + f32(0.5)
    mind2 = np.empty(M, np.float32)
    qx, qy = query_points[:, 0], query_points[:, 1]
    B = 16384
    for i in range(0, M, B):
        dx = qx[i : i + B, None] - px[None, :]
        dy = qy[i : i + B, None] - py[None, :]
        mind2[i : i + B] = (dx * dx + dy * dy).min(axis=1)
    hot = np.flatnonzero(mind2 < R12)
    cold = np.flatnonzero(mind2 >= R12)

    # deal hot queries round-robin so every shard has the same exact-chunk count
    ne = int(np.ceil(max(1, -(-len(hot) // NCORES)) / CH))
    assert ne <= GROUP, f"too many exact chunks: {ne}"
    shard_idx = np.empty((NCORES, MS), np.int64)
    cold_pos = 0
    for i in range(NCORES):
        h = hot[i::NCORES]
        pad = ne * CH - len(h)
        shard = np.concatenate([h, cold[cold_pos : cold_pos + pad]])
        cold_pos += pad
        rest = MS - len(shard)
        shard = np.concatenate([shard, cold[cold_pos : cold_pos + rest]])
        cold_pos += rest
        shard_idx[i] = shard
    assert cold_pos == len(cold)

    params = np.zeros((1, 8), np.float32)
    params[0, :6] = [axv, ayv, txv, tyv, sgv, btv]
    pos_flat = positions.reshape(-1).copy()

    run = _get_runner(ne)
    in_maps = []
    for i in range(NCORES):
        in_maps.append({
            "q": query_points[shard_idx[i]].copy(),
            "positions": pos_flat,
            "activities": activities.copy(),
            "params": params,
        })
    results = run(in_maps)
    _bench_state["run"] = run
    _bench_state["in_maps"] = in_maps

    out = np.empty((M, 3), np.float32)
    for i in range(NCORES):
        out[shard_idx[i]] = results[i]["out"].T
    return out
